# revision 51
# baseline (speedup 1.0000x reference)
"""Distributed MHA forward for trn2 (8 NeuronCores).

Problem: x[4,2048,1024] -> MHA(16 heads, dh=64) -> [4,2048,1024], fp32 I/O.

Sharding: core (b, g) = batch b (4) x head-group g (2 groups of 8 heads).
Each core computes q,k,v projections for its 8 heads, attention, and the
partial out-projection ctx_g @ Wo[g*512:(g+1)*512, :].  Pair-wise
ReduceScatters {2b, 2b+1} sum the partials, one per row chunk (the last
block split in three), each issued right after its rows finish so the
collective overlaps the next block's compute; rank g keeps the g-th half of
every chunk's rows and the host remaps.  Host adds the (bv @ Wo + bo) bias
(bv folds out of attention because softmax rows sum to 1).  bk is dropped
entirely: its score contribution q_i.bk is constant along the softmax axis.
The 1/8 score scale is split sqrt(1/8) onto each of Wq and Wk host-side;
bq rides the PSUM->SBUF evacuation as a per-partition tensor_scalar add.

Device layout (per core, fp32 PSUM accumulation):
  xT   [1024, 2048] = x[b].T bf16     (host-transposed)
  qT,kT [512, 2048] bf16 via lhsT=Wq-slice, rhs=xT (features on partitions)
  v    [2048, 8x80] fp8e4m3 natural, per head 64 v-cols + ones col at e=64
                    -> PV matmul emits the softmax denominator for free
  scoresT [nk, nq] per head via lhsT=kT-chunk (K=64), rhs=qT; head pairs at
                    base partitions 0/64 run concurrently on the PE
  exp: ScalarE from 2-bank PSUM groups -> fp8e4m3; ~2.5 of 16 calls per
       block instead run on VectorE via a one-op fp8 Schraudolph
       (uint8 pattern = 8*log2e*s + 56) to relieve the ScalarE bottleneck
  PV:  fp8 DoubleRow matmuls (two 128-chunks per mm), keeping the M=65
       ones-column denominator; ctxT = (v_aug^T @ exps) / den, bf16
  out  = ctxT^T @ Wo-slice          [2048, 1024] fp32 partial -> ReduceScatter
"""

import numpy as np
import ml_dtypes

import concourse.mybir as mybir
import concourse.tile as tile
from concourse import bacc

B, N, D = 4, 2048, 1024
H, DH, P = 16, 64, 128
HG = 8            # heads per core
GF = HG * DH      # 512 features per head-group
KO = D // P       # 8 k-blocks over model dim
FO = GF // P      # 4 feature blocks of the group
NKC = N // P      # 16 nk chunks
NQB = 512         # nq block
NQBS = N // NQB   # 4
NCORES = 8
VE = 80           # per-head stride in v_sb (64 v + 1 ones + pad to x16)

# One-op fp8 Schraudolph exp: an fp8e4m3 bit pattern of a positive value is
# 8*(log2(x)+7) rounded down, so uint8(8*log2e*s + 56) IS fp8(e^s) up to the
# usual piecewise-linear approximation.  One VectorE tensor_scalar per group.
SCHR_A8 = float(8.0 / np.log(2.0))
SCHR_B8 = 56.0

F32 = mybir.dt.float32
BF16 = mybir.dt.bfloat16
FP8 = mybir.dt.float8e4
U8 = mybir.dt.uint8
BF16_NP = ml_dtypes.bfloat16


def _build_nc():
    nc = bacc.Bacc(
        "TRN2",
        target_bir_lowering=False,
        debug=False,
        num_devices=NCORES,
    )
    xT = nc.dram_tensor("xT", [D, N], BF16, kind="ExternalInput")
    wq = nc.dram_tensor("wq", [D, GF], BF16, kind="ExternalInput")
    wk = nc.dram_tensor("wk", [D, GF], BF16, kind="ExternalInput")
    wv = nc.dram_tensor("wv", [D, GF], BF16, kind="ExternalInput")
    wo = nc.dram_tensor("wo", [GF, D], BF16, kind="ExternalInput")
    bqs = nc.dram_tensor("bqs", [GF], F32, kind="ExternalInput")
    out_ext = nc.dram_tensor("out", [N // 2, D], BF16, kind="ExternalOutput")

    with tile.TileContext(nc) as tc:
        _build_body(nc, tc, xT, wq, wk, wv, wo, bqs, out_ext)
    nc.finalize()
    return nc


def _build_body(nc, tc, xT, wq, wk, wv, wo, bqs, out_ext):
    mm = nc.tensor.matmul
    Exp = mybir.ActivationFunctionType.Exp
    DR = mybir.MatmulPerfMode.DoubleRow

    with (
        tc.tile_pool(name="persist", bufs=1) as pers,
        tc.tile_pool(name="dram", bufs=1, space="DRAM") as dram,
    ):
        # per-fblock q/k tiles so attention on head pair 0 can start while
        # later fblocks are still projecting
        qT_f = [pers.tile([P, N], BF16, name=f"qT_f{fc}") for fc in range(FO)]
        kT_f = [pers.tile([P, N], BF16, name=f"kT_f{fc}") for fc in range(FO)]
        v_sb = pers.tile([P, NKC, HG * VE], FP8, name="v_sb")
        ctxT_sb = pers.tile([P, FO, N], BF16, name="ctxT_sb")
        wo_sb = pers.tile([P, FO, D], BF16, name="wo_sb")
        bq_sb = pers.tile([P, FO], F32, name="bq_sb")

        # One RS chunk per row block: the pair splits each chunk's rows
        # (rank0 keeps the first half, rank1 the second); the host remaps.
        # Each chunk's ReduceScatter issues right after its out-projection and
        # overlaps the next block's compute.  The last block is split in three
        # so only a ~eighth-size collective remains exposed at the end.
        chunk_rows = [NQB, NQB, NQB, NQB // 2, NQB // 4, NQB // 4]
        chunks = [
            dram.tile([r, D], BF16, name=f"chunk{q}")
            for q, r in enumerate(chunk_rows)
        ]
        rs_outs = [
            dram.tile([r // 2, D], BF16, name=f"rs{q}")
            for q, r in enumerate(chunk_rows)
        ]
        recip_drams = [
            [dram.tile([2, NQB], F32, name=f"recip_d{q}_{hp}") for hp in range(4)]
            for q in range(NQBS)
        ]

        nc.sync.dma_start(bq_sb[:], bqs.rearrange("(fo p) -> p fo", p=P))

        # Attention-scope pools are opened first; the phase-1 pools live in a
        # nested ExitStack that closes after the last projection so the
        # out-projection PSUM pool can reuse those banks.
        from contextlib import ExitStack

        att_es = ExitStack()
        exps_pool = att_es.enter_context(tc.tile_pool(name="exps", bufs=2))
        psum_s = att_es.enter_context(tc.tile_pool(name="ps_sc", bufs=2, space="PSUM"))
        psum_pv = att_es.enter_context(tc.tile_pool(name="ps_pv", bufs=2, space="PSUM"))
        misc = att_es.enter_context(tc.tile_pool(name="att_misc", bufs=4))
        outsb = att_es.enter_context(tc.tile_pool(name="out_sb", bufs=4))

        v4 = v_sb.rearrange("p nk (h e) -> p nk h e", e=VE)

        def alloc_exps():
            return [
                exps_pool.tile(
                    [P, NKC, NQB], FP8, tag="exps", name=f"exps{i}", bufs=6
                )
                for i in range(2)
            ]

        def scores_exp(nqb, hp, exps=None, grps=None):
            """scoresT + exp into fp8 exps tiles for one (row block, pair)."""
            qsl = slice(nqb * NQB, (nqb + 1) * NQB)
            if exps is None:
                exps = alloc_exps()
            # 2 nk-chunks per PSUM group; head pair interleaved so the K=64
            # matmuls pack PE row groups 0/64
            for grp in grps if grps is not None else range(NKC // 2):
                pss = [
                    psum_s.tile([P, 2 * NQB], F32, tag="sc", name="ps_sc")
                    for _ in range(2)
                ]
                for j in range(2):
                    nkc = grp * 2 + j
                    ksl = slice(nkc * P, (nkc + 1) * P)
                    for i in range(2):
                        rows = slice(i * 64, (i + 1) * 64)
                        mm(
                            pss[i][:, j * NQB : (j + 1) * NQB],
                            kT_f[hp][rows, ksl],
                            qT_f[hp][rows, qsl],
                            start=True,
                            stop=True,
                        )
                for i in range(2):
                    # offload ~2.5 of 16 exp calls per block to VectorE via
                    # the one-op fp8 Schraudolph; the rest run on ScalarE
                    if grp == 3 or (grp == 6 and i == 0 and (nqb + hp) % 2 == 0):
                        nc.vector.tensor_scalar(
                            exps[i][:, grp * 2 : grp * 2 + 2, :].bitcast(U8),
                            pss[i].rearrange("p (c n) -> p c n", n=NQB),
                            SCHR_A8,
                            SCHR_B8,
                            mybir.AluOpType.mult,
                            mybir.AluOpType.add,
                        )
                    else:
                        nc.scalar.activation(
                            exps[i][:, grp * 2 : grp * 2 + 2, :],
                            pss[i].rearrange("p (c n) -> p c n", n=NQB),
                            Exp,
                        )
            return exps

        def pv_norm(nqb, hp, exps):
            """PV + unnormalized ctx evac + per-pair normalization.

            PV: ctxT_aug[65, nq] per head via fp8 DoubleRow (2 nk chunks per
            matmul); den lands on PSUM partition 64 via the ones column."""
            qsl = slice(nqb * NQB, (nqb + 1) * NQB)
            for i in range(2):
                hl = 2 * hp + i
                ps_pv = psum_pv.tile([DH + 1, NQB], F32, tag="pv", name="ps_pv")
                for nkp in range(NKC // 2):
                    mm(
                        ps_pv[:],
                        v4[:, 2 * nkp : 2 * nkp + 2, hl, 0 : DH + 1],
                        exps[i][:, 2 * nkp : 2 * nkp + 2, :],
                        start=(nkp == 0),
                        stop=(nkp == NKC // 2 - 1),
                        perf_mode=DR,
                    )
                if i == 0:
                    nc.vector.tensor_copy(ctxT_sb[0:64, hp, qsl], ps_pv[0:DH, :])
                else:
                    ctmp = misc.tile([64, NQB], BF16, tag="ctmp", name="ctmp", bufs=2)
                    nc.vector.tensor_copy(ctmp[:], ps_pv[0:DH, :])
                    nc.sync.dma_start(ctxT_sb[64:128, hp, qsl], ctmp[:])
                # reciprocal straight off PSUM partition 64 (same lane), then
                # DMA the row into DRAM so it can be partition-broadcast back
                rstage = misc.tile([65, NQB], F32, tag="dstage", name="rstage", bufs=2)
                nc.vector.reciprocal(rstage[64:65, :], ps_pv[64:65, :])
                nc.sync.dma_start(recip_drams[nqb][hp][i : i + 1, :], rstage[64:65, :])
            # normalize this pair in place; overlaps the next pair's compute
            qsl2 = qsl
            rd = recip_drams[nqb][hp]
            rbc = misc.tile([P, NQB], F32, tag="rbc", name="rbc", bufs=2)
            nc.sync.dma_start(rbc[0:64, :], rd[0:1, :].to_broadcast((64, NQB)))
            nc.sync.dma_start(rbc[64:128, :], rd[1:2, :].to_broadcast((64, NQB)))
            nc.vector.tensor_tensor(
                ctxT_sb[:, hp, qsl2],
                ctxT_sb[:, hp, qsl2],
                rbc[:],
                mybir.AluOpType.mult,
            )

        def finish_block(nqb, psum_o):
            """out projection of one row block into its RS chunk(s)."""
            for lq in range(NQB // P):
                nqc = nqb * (NQB // P) + lq
                if nqb < NQBS - 1:
                    chunk, crow = chunks[nqb], lq * P
                elif lq < 2:
                    chunk, crow = chunks[3], lq * P
                elif lq == 2:
                    chunk, crow = chunks[4], 0
                else:
                    chunk, crow = chunks[5], 0
                for cb in range(D // NQB):
                    ps = psum_o.tile([P, NQB], F32, tag="o", name="ps_o")
                    for fc in range(FO):
                        mm(
                            ps[:],
                            ctxT_sb[:, fc, nqc * P : (nqc + 1) * P],
                            wo_sb[:, fc, cb * NQB : (cb + 1) * NQB],
                            start=(fc == 0),
                            stop=(fc == FO - 1),
                        )
                    ob = outsb.tile([P, NQB], BF16, tag="ob", name="ob", bufs=2)
                    nc.vector.tensor_copy(ob[:], ps[:])
                    nc.sync.dma_start(
                        chunk[crow : crow + P, cb * NQB : (cb + 1) * NQB],
                        ob[:],
                    )
                if nqb == NQBS - 1:
                    if lq == 1:
                        reduce_scatter(3)
                    elif lq == 2:
                        reduce_scatter(4)

        rs_row0 = [0, 256, 512, 768, 896, 960]

        def reduce_scatter(q):
            nc.gpsimd.collective_compute(
                "ReduceScatter",
                mybir.AluOpType.add,
                replica_groups=[[0, 1], [2, 3], [4, 5], [6, 7]],
                ins=[chunks[q].opt()],
                outs=[rs_outs[q].opt()],
            )
            # gpsimd queue: a sync-queue DMA here would head-of-line block all
            # later sync DMAs behind the RS wait
            nc.gpsimd.dma_start(
                out_ext[rs_row0[q] : rs_row0[q] + chunk_rows[q] // 2, :],
                rs_outs[q][:],
            )

        # ---- Phase 1 (projections), interleaved with block-0 attention ----
        ph1_es = ExitStack()
        ph1 = ph1_es.enter_context(tc.tile_pool(name="ph1", bufs=1))
        psum1 = ph1_es.enter_context(
            tc.tile_pool(name="ph1_psum", bufs=2, space="PSUM")
        )
        # per-ko weight/xT DMA slices: the first projection matmul only waits
        # for the first pieces instead of multi-MB whole-tensor loads
        wk_sb = ph1.tile([P, KO, GF], BF16, name="wk_sb")
        wq_sb = ph1.tile([P, KO, GF], BF16, name="wq_sb")
        wv_sb = ph1.tile([P, KO, GF], BF16, name="wv_sb")
        xT4 = xT.rearrange("(ko p) n -> ko p n", p=P)
        wk4 = wk.rearrange("(ko p) f -> ko p f", p=P)
        wq4 = wq.rearrange("(ko p) f -> ko p f", p=P)
        wv4 = wv.rearrange("(ko p) f -> ko p f", p=P)
        xT_k = [ph1.tile([P, N], BF16, name=f"xT_k{ko}") for ko in range(KO)]
        for ko in range(KO):
            nc.sync.dma_start(wk_sb[:, ko], wk4[ko])
            nc.sync.dma_start(xT_k[ko][:], xT4[ko])
        for ko in range(KO):
            nc.sync.dma_start(wq_sb[:, ko], wq4[ko])
        for ko in range(KO):
            nc.sync.dma_start(wv_sb[:, ko], wv4[ko])
        nc.sync.dma_start(wo_sb[:], wo.rearrange("(fo p) d -> p fo d", p=P))

        def proj_qk(fc):
            # bq rides the PSUM evacuation as a per-partition add; bk is
            # dropped (its score term is constant along the softmax axis)
            for w_sb, dst, bias in (
                (wk_sb, kT_f[fc], None),
                (wq_sb, qT_f[fc], bq_sb),
            ):
                for nb in range(NQBS):
                    ps = psum1.tile([P, NQB], F32, tag="proj", name="ps_proj")
                    for ko in range(KO):
                        mm(
                            ps[:],
                            w_sb[:, ko, fc * P : (fc + 1) * P],
                            xT_k[ko][:, nb * NQB : (nb + 1) * NQB],
                            start=(ko == 0),
                            stop=(ko == KO - 1),
                        )
                    if bias is None:
                        nc.vector.tensor_copy(
                            dst[:, nb * NQB : (nb + 1) * NQB], ps[:]
                        )
                    else:
                        nc.vector.tensor_scalar(
                            dst[:, nb * NQB : (nb + 1) * NQB],
                            ps[:],
                            bias[:, fc : fc + 1],
                            None,
                            mybir.AluOpType.add,
                        )

        # The first two pairs' scores/exp issue before the v projection so the
        # ScalarE exp stream starts as soon as the first q/k tiles exist and
        # is not starved while the PE runs the v projection.
        proj_qk(0)
        ex0 = scores_exp(0, 0)
        proj_qk(1)
        ex1 = scores_exp(0, 1)
        proj_qk(2)
        ex2 = scores_exp(0, 2)
        # v natural: [n, 512] = xT-chunk^T @ Wv, fp8 with a ones col at e=64
        nc.vector.memset(v4[:, :, :, DH], 1.0)
        for nk in range(NKC):
            ps = psum1.tile([P, GF], F32, tag="proj", name="ps_v")
            for ko in range(KO):
                mm(
                    ps[:],
                    xT_k[ko][:, nk * P : (nk + 1) * P],
                    wv_sb[:, ko, :],
                    start=(ko == 0),
                    stop=(ko == KO - 1),
                )
            nc.vector.tensor_copy(
                v4[:, nk, :, :DH],
                ps.rearrange("p (h e) -> p h e", e=DH),
            )

        pv_norm(0, 0, ex0)
        proj_qk(3)
        ex3 = scores_exp(0, 3)
        pv_norm(0, 1, ex1)
        pv_norm(0, 2, ex2)
        pv_norm(0, 3, ex3)
        ph1_es.close()

        # out-projection PSUM pool opens after the projection pool closed
        psum_o = att_es.enter_context(
            tc.tile_pool(name="out_ps", bufs=2, space="PSUM")
        )
        # one-pair score/exp lookahead across block boundaries: the next
        # block's first pair is issued before each finish_block so the exp
        # stream is never starved during the out-projection matmuls
        exn = scores_exp(1, 0)
        finish_block(0, psum_o)
        reduce_scatter(0)

        for nqb in range(1, NQBS):
            for hp in range(HG // 2):
                pv_norm(nqb, hp, exn)
                if hp + 1 < HG // 2:
                    exn = scores_exp(nqb, hp + 1)
                elif nqb + 1 < NQBS:
                    exn = scores_exp(nqb + 1, 0)
            finish_block(nqb, psum_o)
            reduce_scatter(nqb if nqb < NQBS - 1 else 5)
        att_es.close()


_NC_CACHE = None


def _get_nc():
    global _NC_CACHE
    if _NC_CACHE is None:
        _NC_CACHE = _build_nc()
    return _NC_CACHE


# --------------------------------------------------------------------------
# Timing support (test-only): build the sharded jit once, rerun on
# device-resident inputs, and subtract the axon dispatch floor measured on a
# trivial kernel.
# --------------------------------------------------------------------------


def _build_trivial_nc():
    nc = bacc.Bacc("TRN2", target_bir_lowering=False, debug=False,
                   num_devices=NCORES)
    tin = nc.dram_tensor("tin", [P, P], F32, kind="ExternalInput")
    tout = nc.dram_tensor("tout", [P, P], F32, kind="ExternalOutput")
    with tile.TileContext(nc) as tc:
        with tc.tile_pool(name="t", bufs=1) as pool:
            t = pool.tile([P, P], F32, name="t")
            nc.sync.dma_start(t[:], tin[:, :])
            nc.sync.dma_start(tout[:, :], t[:])
    nc.finalize()
    return nc


class _SpmdRunner:
    """Mirror of bass2jax.run_bass_via_pjrt's multi-core path with a cached
    jit so repeat executions don't recompile."""

    def __init__(self, nc):
        import jax
        from jax.sharding import Mesh, PartitionSpec
        try:
            from jax.experimental.shard_map import shard_map
        except ImportError:
            from jax.shard_map import shard_map
        from concourse import bass2jax as b2j

        b2j.install_neuronx_cc_hook()
        self.nc = nc
        partition_name = (
            nc.partition_id_tensor.name if nc.partition_id_tensor else None
        )
        in_names, out_names, out_avals, zero_outs = [], [], [], []
        for alloc in nc.m.functions[0].allocations:
            if not isinstance(alloc, mybir.MemoryLocationSet):
                continue
            name = alloc.memorylocations[0].name
            if alloc.kind == "ExternalInput":
                if name != partition_name:
                    in_names.append(name)
            elif alloc.kind == "ExternalOutput":
                shape = tuple(alloc.tensor_shape)
                dtype = mybir.dt.np(alloc.dtype)
                out_names.append(name)
                out_avals.append(jax.core.ShapedArray(shape, dtype))
                zero_outs.append(np.zeros(shape, dtype))
        self.n_params = len(in_names)
        n_outs = len(out_avals)
        in_names = in_names + out_names
        if partition_name is not None:
            in_names.append(partition_name)
        self.in_names = in_names
        self.out_names = out_names
        self.out_avals = out_avals
        self.zero_outs = zero_outs

        def _body(*args):
            operands = list(args)
            if partition_name is not None:
                operands.append(b2j.partition_id_tensor())
            outs = b2j._bass_exec_p.bind(
                *operands,
                out_avals=tuple(out_avals),
                in_names=tuple(in_names),
                out_names=tuple(out_names),
                lowering_input_output_aliases=(),
                sim_require_finite=True,
                sim_require_nnan=True,
                nc=nc,
            )
            return tuple(outs)

        devices = jax.devices()[:NCORES]
        self.mesh = Mesh(np.asarray(devices), ("core",))
        in_specs = (PartitionSpec("core"),) * (self.n_params + n_outs)
        out_specs = (PartitionSpec("core"),) * n_outs
        self.fn = jax.jit(
            shard_map(_body, mesh=self.mesh, in_specs=in_specs,
                      out_specs=out_specs, check_rep=False),
            keep_unused=True,
        )
        self._jax = jax

    def make_fn_k(self, K):
        """jit that executes the NEFF K times; all outputs kept live so the
        calls can't be DCE'd. Used to amortize the ~78 ms axon dispatch floor
        out of timing: exec_ns ~= (t(K) - t(1)) / (K - 1)."""
        import jax
        from jax.sharding import PartitionSpec
        try:
            from jax.experimental.shard_map import shard_map
        except ImportError:
            from jax.shard_map import shard_map
        from concourse import bass2jax as b2j

        nc = self.nc
        partition_name = nc.partition_id_tensor.name if nc.partition_id_tensor else None
        in_names, out_names, out_avals = self.in_names, self.out_names, self.out_avals

        def _body_k(*args):
            all_outs = []
            for _ in range(K):
                operands = list(args)
                if partition_name is not None:
                    operands.append(b2j.partition_id_tensor())
                outs = b2j._bass_exec_p.bind(
                    *operands,
                    out_avals=tuple(out_avals),
                    in_names=tuple(in_names),
                    out_names=tuple(out_names),
                    lowering_input_output_aliases=(),
                    sim_require_finite=True,
                    sim_require_nnan=True,
                    nc=nc,
                )
                all_outs.extend(outs)
            return tuple(all_outs)

        n_outs = len(out_avals)
        in_specs = (PartitionSpec("core"),) * (self.n_params + n_outs)
        out_specs = (PartitionSpec("core"),) * (n_outs * K)
        return jax.jit(
            shard_map(_body_k, mesh=self.mesh, in_specs=in_specs,
                      out_specs=out_specs, check_rep=False),
            keep_unused=True,
        )

    def time_k(self, in_maps, K=8, reps=12):
        import time as _time

        dev_in, dev_zero = self.prepare(in_maps)
        fn_k = self.make_fn_k(K)
        fn_1 = self.make_fn_k(1)
        for fn in (fn_1, fn_k):
            self._jax.block_until_ready(fn(*dev_in, *dev_zero))  # compile+warm
        t1s, tks = [], []
        for _ in range(reps):
            t0 = _time.perf_counter()
            self._jax.block_until_ready(fn_1(*dev_in, *dev_zero))
            t1s.append(_time.perf_counter() - t0)
            t0 = _time.perf_counter()
            self._jax.block_until_ready(fn_k(*dev_in, *dev_zero))
            tks.append(_time.perf_counter() - t0)
        t1, tk = min(t1s), min(tks)
        return (tk - t1) / (K - 1), t1, tk

    def _shard(self, arrs):
        import jax
        from jax.sharding import NamedSharding, PartitionSpec

        sh = NamedSharding(self.mesh, PartitionSpec("core"))
        return [jax.device_put(a, sh) for a in arrs]

    def prepare(self, in_maps):
        concat_in = [
            np.concatenate([np.asarray(m[name]) for m in in_maps], axis=0)
            for name in self.in_names[: self.n_params]
        ]
        concat_zeros = [
            np.zeros((NCORES * z.shape[0], *z.shape[1:]), z.dtype)
            for z in self.zero_outs
        ]
        return self._shard(concat_in), self._shard(concat_zeros)

    def run(self, dev_in, dev_zero):
        outs = self.fn(*dev_in, *dev_zero)
        self._jax.block_until_ready(outs)
        return outs

    def time(self, in_maps, reps=10):
        import time as _time

        dev_in, dev_zero = self.prepare(in_maps)
        self.run(dev_in, dev_zero)  # warm/compile
        ts = []
        for _ in range(reps):
            t0 = _time.perf_counter()
            self.run(dev_in, dev_zero)
            ts.append(_time.perf_counter() - t0)
        return min(ts), ts

    def results(self, in_maps):
        dev_in, dev_zero = self.prepare(in_maps)
        outs = self.run(dev_in, dev_zero)
        res = []
        for c in range(NCORES):
            res.append(
                {
                    name: np.asarray(outs[i]).reshape(
                        NCORES, *self.out_avals[i].shape
                    )[c]
                    for i, name in enumerate(self.out_names)
                }
            )
        return res


_RUNNER = None
_TRIVIAL_RUNNER = None


def get_runner():
    global _RUNNER
    if _RUNNER is None:
        _RUNNER = _SpmdRunner(_get_nc())
    return _RUNNER


def get_trivial_runner():
    global _TRIVIAL_RUNNER
    if _TRIVIAL_RUNNER is None:
        _TRIVIAL_RUNNER = _SpmdRunner(_build_trivial_nc())
    return _TRIVIAL_RUNNER


def make_in_maps(x, Wq, bq, Wk, bk, Wv, bv, Wo, bo):
    x = np.asarray(x, np.float32)
    s = np.float32(1.0 / (2.0 * np.sqrt(2.0)))  # sqrt(1/8) on each of q, k
    in_maps = []
    for core in range(NCORES):
        b, g = core // 2, core % 2
        gsl = slice(g * GF, (g + 1) * GF)
        in_maps.append(
            {
                "xT": np.ascontiguousarray(x[b].T).astype(BF16_NP),
                "wq": np.ascontiguousarray(np.asarray(Wq)[:, gsl] * s).astype(
                    BF16_NP
                ),
                "wk": np.ascontiguousarray(np.asarray(Wk)[:, gsl] * s).astype(
                    BF16_NP
                ),
                "wv": np.ascontiguousarray(np.asarray(Wv)[:, gsl]).astype(BF16_NP),
                "wo": np.ascontiguousarray(np.asarray(Wo)[gsl, :]).astype(BF16_NP),
                "bqs": np.ascontiguousarray(
                    np.asarray(bq)[gsl].astype(np.float32) * s
                ),
            }
        )
    return in_maps


def kernel(x, Wq, bq, Wk, bk, Wv, bv, Wo, bo):
    Wo = np.asarray(Wo, np.float32)
    bv = np.asarray(bv, np.float32)
    bo = np.asarray(bo, np.float32)
    in_maps = make_in_maps(x, Wq, bq, Wk, bk, Wv, bv, Wo, bo)
    results = get_runner().results(in_maps)

    post = (bv @ Wo + bo).astype(np.float32)  # softmax rows sum to 1 -> bv folds
    # Per-chunk RS: rank g keeps the g-th half of each chunk's rows.
    # Segments: (global row start for rank 0, rows per rank).
    segments = [(0, 256), (512, 256), (1024, 256), (1536, 128), (1792, 64), (1920, 64)]
    out = np.empty((B, N, D), np.float32)
    for b in range(B):
        for g in range(2):
            r = np.asarray(results[2 * b + g]["out"], np.float32)
            ofs = 0
            for g0, nrows in segments:
                out[b, g0 + g * nrows : g0 + (g + 1) * nrows] = r[ofs : ofs + nrows]
                ofs += nrows
        out[b] += post
    return out

